# revision 58
# baseline (speedup 1.0000x reference)
"""Trainium2 Bass kernel for DualAttention (general+pos scorer, renormalized).

Contract: kernel(**inputs) takes FULL unsharded numpy inputs and returns the
full outputs (attn_h, av, ma, pa) matching reference().

Strategy (data-parallel over batch, 8 batches per core on 8 cores):
  - q_b = W_main_in.T @ s_b computed on PE as a broadcast matmul (step-0
    free-dim lhsT replicates s_b across all 128 output partitions).
  - main_align = me_b @ q_b : fused multiply+accumulate on DVE
    (scalar_tensor_tensor with accum_out) while streaming me once (64MB).
  - pos_align = pe_b @ qp_b : DVE mult + 3D reduce over the pos stream.
  - softmax: DVE rowmax, GPSIMD partition_all_reduce (max/sum broadcast),
    ACT exp with accum_out.  av = exp(m+p - Mm - Mp)/Z (renorm identity).
  - c_t = sum_l avu[l]*me[l] on PE (K=128 l's, N=512, float32r = 1 cyc/row)
    from SBUF-resident me, using UNNORMALIZED weights so the matmuls don't
    wait for the global sum; c_t is scaled by 1/Z during PSUM evacuation.
  - attn_h = tanh(W_out @ [c_t; s]) with host-transposed W_out, batched over
    the core's 8 batches in two halves.
Host side does only sharding and pure relayouts (transposes, additive masks).
"""

import os
import sys

for _p in ("/opt/trn_rl_repo", "/root/.axon_site/_ro/trn_rl_repo"):
    if os.path.isdir(_p) and _p not in sys.path:
        sys.path.insert(0, _p)

import numpy as np

B, L, D, DP = 64, 4096, 512, 64
NCORES = 8
BL = B // NCORES          # local batches per core
P = 128                   # partitions
NT = L // P               # 32 l-tiles
NCH = 4                   # me chunks per batch
TCH = NT // NCH           # 8 l-tiles per me chunk
PCH = 8                   # pos chunks
PT = NT // PCH            # 4 l-tiles per pos chunk

CTX_DTYPE = os.environ.get("CTX_DTYPE", "float32r")  # float32 | float32r

_CACHE = {}


def _build():
    import concourse.bass as bass
    import concourse.bacc as bacc
    import concourse.tile as tile
    from concourse import mybir
    from concourse import bass_isa

    f32 = mybir.dt.float32
    ctx_dt = getattr(mybir.dt, CTX_DTYPE)

    nc = bacc.Bacc("TRN2", target_bir_lowering=False, debug=False)

    me_d = nc.dram_tensor("me", [L, BL, D], f32, kind="ExternalInput").ap()
    pos_d = nc.dram_tensor("pos", [L, BL, DP], f32, kind="ExternalInput").ap()
    srcT_d = nc.dram_tensor("srcT", [P, D // P, BL], f32, kind="ExternalInput").ap()
    wmain_d = nc.dram_tensor("wmain", [D, D], f32, kind="ExternalInput").ap()
    wpos_d = nc.dram_tensor("wpos", [D, DP], f32, kind="ExternalInput").ap()
    woutT_d = nc.dram_tensor("woutT", [2 * D, D], f32, kind="ExternalInput").ap()
    mask_d = nc.dram_tensor("mask2", [P, BL, 2, NT], f32, kind="ExternalInput").ap()

    oah_d = nc.dram_tensor("out_ah", [P, D // P, BL], f32, kind="ExternalOutput").ap()
    oma_d = nc.dram_tensor("out_ma", [P, BL, NT], f32, kind="ExternalOutput").ap()
    opa_d = nc.dram_tensor("out_pa", [P, BL, NT], f32, kind="ExternalOutput").ap()
    oav_d = nc.dram_tensor("out_av", [P, BL, NT], f32, kind="ExternalOutput").ap()

    AF = mybir.ActivationFunctionType
    AL = mybir.AluOpType
    AX = mybir.AxisListType

    # DRAM views: l = (c, t, p) row-major split
    me_r = me_d.rearrange("(c t p) b d -> c b p t d", p=P, t=TCH)
    pos_r = pos_d.rearrange("(c t p) b d -> c p t b d", p=P, t=PT)
    wm_r = wmain_d.rearrange("(h p) d -> h p d", p=P)
    wp_r = wpos_d.rearrange("(h p) d -> p h d", p=P)
    wo_r = woutT_d.rearrange("(h p) i -> p h i", p=P)

    with tile.TileContext(nc) as tc:
        with (
            tc.tile_pool(name="init", bufs=1) as init,
            tc.tile_pool(name="mep", bufs=7) as mep,
            tc.tile_pool(name="posp", bufs=3) as posp,
            tc.tile_pool(name="small", bufs=3) as small,
            tc.tile_pool(name="misc", bufs=2) as misc,
            tc.tile_pool(name="dump", bufs=3) as dump,
            tc.tile_pool(name="ps_tp", bufs=2, space="PSUM") as ps_tp,
            tc.tile_pool(name="ps_ctx", bufs=2, space="PSUM") as ps_ctx,
        ):
            # ---------- boot ----------
            st = init.tile([P, D // P, BL], f32, tag="st")
            nc.sync.dma_start(out=st, in_=srcT_d)
            # W_main in 4 separate chunk loads so the first query matmul can
            # start after 256KB instead of 1MB
            wmc = []
            for h in range(D // P):
                w1 = init.tile([P, D], f32, tag=f"wm{h}")
                nc.sync.dma_start(out=w1, in_=wm_r[h])
                wmc.append(w1)
            wp_t = init.tile([P, D // P, DP], f32, tag="wp")
            nc.sync.dma_start(out=wp_t, in_=wp_r)

            ones1 = init.tile([1, P], f32, tag="ones1")
            nc.vector.memset(ones1, 1.0)
            ident1 = init.tile([1, 1], f32, tag="ident1")
            nc.vector.memset(ident1, 1.0)

            QB = init.tile([P, BL, D], f32, tag="qb")
            QPB = init.tile([P, BL, DP], f32, tag="qpb")
            ALP = init.tile([P, BL, NT], f32, tag="alp")
            MAo = init.tile([P, BL, NT], f32, tag="mao")
            PAo = init.tile([P, BL, NT], f32, tag="pao")
            AVo = init.tile([P, BL, NT], f32, tag="avo")
            CCT = init.tile([P, D // P, BL], f32, tag="cct")
            AH = init.tile([P, D // P, BL], f32, tag="ah")

            with tc.tile_pool(name="ps_boot", bufs=2, space="PSUM") as psb:
                # warm the PE (HAM clock gate releases after ~3.4us of
                # sustained activity) with dep-free dummy matmuls
                wrm = psb.tile([1, P], f32, tag="bb")
                for _ in range(6):
                    nc.tensor.matmul(wrm, lhsT=ones1[:, 0:1], rhs=ones1,
                                     start=True, stop=True)

                # per-batch broadcast queries: lhsT = s_b column with a
                # step-0 free dim -> QB[p, d] = sum_e s_b[e] W[e, d] for all p
                for b in range(BL):
                    bb_p = psb.tile([P, D], f32, tag="bb")
                    for h in range(D // P):
                        col = st[:, h, b : b + 1]
                        lhsT = bass.AP(tensor=col.tensor, offset=col.offset,
                                       ap=[col.ap[0], [0, P]])
                        nc.tensor.matmul(
                            bb_p, lhsT=lhsT, rhs=wmc[h],
                            start=(h == 0), stop=(h == D // P - 1),
                        )
                    nc.scalar.copy(QB[:, b, :], bb_p)
                    bp_p = psb.tile([P, DP], f32, tag="bp")
                    for h in range(D // P):
                        col = st[:, h, b : b + 1]
                        lhsT = bass.AP(tensor=col.tensor, offset=col.offset,
                                       ap=[col.ap[0], [0, P]])
                        nc.tensor.matmul(
                            bp_p, lhsT=lhsT, rhs=wp_t[:, h, :],
                            start=(h == 0), stop=(h == D // P - 1),
                        )
                    nc.scalar.copy(QPB[:, b, :], bp_p)

            # hoist batch-0 me loads ahead of the pos stream
            b0_mes = []
            for c in range(NCH):
                met = mep.tile([P, TCH, D], ctx_dt, tag="me")
                nc.sync.dma_start(out=met, in_=me_r[c, 0].bitcast(ctx_dt))
                b0_mes.append(met)

            # ---------- pos aligns for all batches (stream pos once) -------
            for pc in range(PCH):
                pt = posp.tile([P, PT, BL, DP], f32, tag="pt")
                nc.sync.dma_start(out=pt, in_=pos_r[pc])
                for t in range(PT):
                    nc.vector.tensor_tensor(
                        out=pt[:, t, :, :], in0=pt[:, t, :, :], in1=QPB,
                        op=AL.mult,
                    )
                out_ap = ALP[:, :, pc * PT : (pc + 1) * PT].rearrange(
                    "p b t -> p t b"
                )
                nc.vector.tensor_reduce(out=out_ap, in_=pt, axis=AX.X, op=AL.add)

            # tail-only / later-needed loads, emitted after the hot boot path
            wt = init.tile([P, 2 * D // P, D], f32, tag="wt")
            nc.sync.dma_start(out=wt, in_=wo_r)
            mask_t = init.tile([P, BL, 2, NT], f32, tag="mask")
            nc.sync.dma_start(out=mask_t, in_=mask_d)

            # ---------- per-batch: main align, softmax, context ----------
            for b in range(BL):
                mes = []
                alm = small.tile([P, NT], f32, tag="alm")
                for c in range(NCH):
                    if b == 0:
                        met = b0_mes[c]
                    else:
                        met = mep.tile([P, TCH, D], ctx_dt, tag="me")
                        nc.sync.dma_start(out=met, in_=me_r[c, b].bitcast(ctx_dt))
                    mes.append(met)
                    for t in range(TCH):
                        dmp = dump.tile([P, D], f32, tag="dmp")
                        gt = c * TCH + t
                        nc.vector.scalar_tensor_tensor(
                            out=dmp, in0=met[:, t, :].bitcast(f32), scalar=1.0,
                            in1=QB[:, b, :], op0=AL.mult, op1=AL.mult,
                            accum_out=alm[:, gt : gt + 1],
                        )

                # masked scores: am[:,0,:]=main, am[:,1,:]=pos
                am = small.tile([P, 2, NT], f32, tag="am")
                nc.vector.tensor_tensor(out=am[:, 0, :], in0=alm,
                                        in1=mask_t[:, b, 0, :], op=AL.add)
                nc.vector.tensor_tensor(out=am[:, 1, :], in0=ALP[:, b, :],
                                        in1=mask_t[:, b, 1, :], op=AL.add)
                m2 = small.tile([P, 2], f32, tag="m2")
                nc.vector.tensor_reduce(out=m2, in_=am, axis=AX.X, op=AL.max)
                g2 = small.tile([P, 2], f32, tag="g2")
                nc.gpsimd.partition_all_reduce(
                    g2, m2, channels=P, reduce_op=bass_isa.ReduceOp.max
                )
                ng2 = small.tile([P, 2], f32, tag="ng2")
                nc.scalar.mul(ng2, g2, -1.0)

                ex = small.tile([P, 2, NT], f32, tag="ex")
                s3 = small.tile([P, 3], f32, tag="s3")
                nc.scalar.activation(
                    out=ex[:, 0, :], in_=am[:, 0, :], func=AF.Exp,
                    bias=ng2[:, 0:1], scale=1.0, accum_out=s3[:, 0:1],
                )
                nc.scalar.activation(
                    out=ex[:, 1, :], in_=am[:, 1, :], func=AF.Exp,
                    bias=ng2[:, 1:2], scale=1.0, accum_out=s3[:, 1:2],
                )
                avu = small.tile([P, NT], f32, tag="avu")
                nc.vector.scalar_tensor_tensor(
                    out=avu, in0=ex[:, 0, :], scalar=1.0, in1=ex[:, 1, :],
                    op0=AL.mult, op1=AL.mult, accum_out=s3[:, 2:3],
                )
                # context weights: unnormalized avu rounded to the context
                # dtype; normalization folded into the PSUM evacuation below
                avr = small.tile([P, NT], ctx_dt, tag="avr")
                nc.scalar.copy(avr, avu)

                z3 = small.tile([P, 3], f32, tag="z3")
                nc.gpsimd.partition_all_reduce(
                    z3, s3, channels=P, reduce_op=bass_isa.ReduceOp.add
                )
                r3 = small.tile([P, 3], f32, tag="r3")
                nc.vector.reciprocal(r3, z3)

                nc.scalar.activation(out=MAo[:, b, :], in_=ex[:, 0, :],
                                     func=AF.Copy, scale=r3[:, 0:1])
                nc.sync.dma_start(out=oma_d[:, b, :], in_=MAo[:, b, :])
                nc.scalar.activation(out=PAo[:, b, :], in_=ex[:, 1, :],
                                     func=AF.Copy, scale=r3[:, 1:2])
                nc.sync.dma_start(out=opa_d[:, b, :], in_=PAo[:, b, :])
                nc.scalar.activation(out=AVo[:, b, :], in_=avu,
                                     func=AF.Copy, scale=r3[:, 2:3])
                nc.sync.dma_start(out=oav_d[:, b, :], in_=AVo[:, b, :])

                # context: c_t = (1/Z) * sum_l avu[l] * me[l, :]
                ctx_p = ps_ctx.tile([1, D], f32, tag="ctx")
                for c in range(NCH):
                    for t in range(TCH):
                        gt = c * TCH + t
                        nc.tensor.matmul(
                            ctx_p,
                            lhsT=avr[:, gt : gt + 1],
                            rhs=mes[c][:, t, :],
                            start=(gt == 0), stop=(gt == NT - 1),
                        )
                cts = misc.tile([1, D], f32, tag="cts")
                nc.scalar.activation(out=cts, in_=ctx_p, func=AF.Copy,
                                     scale=r3[0:1, 2:3])
                for k in range(D // P):
                    tp_p = ps_tp.tile([P, 1], f32, tag="tp")
                    nc.tensor.transpose(
                        tp_p, cts[0:1, k * P : (k + 1) * P], ident1
                    )
                    nc.scalar.copy(CCT[:, k, b : b + 1], tp_p)

            # ---------- tail: attn_h = tanh(W_out @ [c_t; s]) ----------
            with tc.tile_pool(name="ps_ah", bufs=3, space="PSUM") as psah:
                for lo, hi in ((0, BL // 2), (BL // 2, BL)):
                    for ic in range(D // P):
                        ah_p = psah.tile([P, hi - lo], f32, tag="ahp")
                        for jc in range(2 * D // P):
                            rhs = (CCT[:, jc, lo:hi] if jc < D // P
                                   else st[:, jc - D // P, lo:hi])
                            nc.tensor.matmul(
                                ah_p, lhsT=wt[:, jc, ic * P : (ic + 1) * P],
                                rhs=rhs,
                                start=(jc == 0), stop=(jc == 2 * D // P - 1),
                            )
                        nc.scalar.activation(out=AH[:, ic, lo:hi], in_=ah_p,
                                             func=AF.Tanh)

            nc.sync.dma_start(out=oah_d, in_=AH)

            # HAM pacemaker: lowest-priority dep-free matmuls that fill PE
            # idle gaps so the clock gate stays at 8/8 (2.4 GHz) for the
            # real context/output matmuls.
            with tc.tile_pool(name="ps_pace", bufs=1, space="PSUM") as pspace:
                pace = pspace.tile([1, D], f32, tag="pace")
                for _ in range(120):
                    nc.tensor.matmul(pace, lhsT=ones1[:, 0:1],
                                     rhs=QB[0:1, 0, :], start=True, stop=True)

    nc.compile()
    return nc


def _get_nc():
    if "nc" not in _CACHE:
        _CACHE["nc"] = _build()
    return _CACHE["nc"]


def _make_in_maps(source, main_embs, pos_embs, W_main_in, W_pos_in, W_out,
                  memory_lengths):
    source = np.asarray(source, np.float32)
    main_embs = np.asarray(main_embs, np.float32)
    pos_embs = np.asarray(pos_embs, np.float32)
    W_main_in = np.ascontiguousarray(np.asarray(W_main_in, np.float32))
    W_pos_in = np.ascontiguousarray(np.asarray(W_pos_in, np.float32))
    woutT = np.ascontiguousarray(np.asarray(W_out, np.float32).T)
    lens = np.asarray(memory_lengths)

    maskadd = np.where(
        np.arange(L)[None, :] < np.asarray(lens, np.int64)[:, None], 0.0, -1e30
    ).astype(np.float32)                      # (B, L)
    m = maskadd.reshape(B, NT, P).transpose(2, 0, 1)  # (P, B, NT)
    mask2 = np.ascontiguousarray(np.stack([m, m], axis=2))  # (P, B, 2, NT)

    in_maps = []
    for c in range(NCORES):
        sl = slice(c * BL, (c + 1) * BL)
        srcT = np.ascontiguousarray(
            source[sl].T.reshape(D // P, P, BL).transpose(1, 0, 2)
        )  # (P, 4, BL): srcT[p,h,b] = source[c*BL+b, h*P+p]
        in_maps.append({
            "me": np.ascontiguousarray(main_embs[:, sl, :]),
            "pos": np.ascontiguousarray(pos_embs[:, sl, :]),
            "srcT": srcT,
            "wmain": W_main_in,
            "wpos": W_pos_in,
            "woutT": woutT,
            "mask2": np.ascontiguousarray(mask2[:, sl]),
        })
    return in_maps


def _assemble(results):
    ah_rows, ma_rows, pa_rows, av_rows = [], [], [], []
    for r in results:
        # out_ah (P, 4, BL): attn[b, h*P+p] = out_ah[p, h, b]
        ah_rows.append(np.ascontiguousarray(
            np.asarray(r["out_ah"]).transpose(2, 1, 0).reshape(BL, D)))
        for rows, key in ((ma_rows, "out_ma"), (pa_rows, "out_pa"),
                          (av_rows, "out_av")):
            # (P, BL, NT): x[b, t*P+p] = arr[p, b, t]
            rows.append(np.ascontiguousarray(
                np.asarray(r[key]).transpose(1, 2, 0).reshape(BL, L)))
    attn_h = np.concatenate(ah_rows, 0)
    ma = np.concatenate(ma_rows, 0)
    pa = np.concatenate(pa_rows, 0)
    av = np.concatenate(av_rows, 0)
    return attn_h, av, ma, pa


def run_hw(inputs, trace=False, **kw):
    from concourse import bass_utils
    nc = _get_nc()
    in_maps = _make_in_maps(**inputs)
    res = bass_utils.run_bass_kernel_spmd(
        nc, in_maps, core_ids=list(range(NCORES)), trace=trace, **kw
    )
    return _assemble(res.results), res


def kernel(source, main_embs, pos_embs, W_main_in, W_pos_in, W_out,
           memory_lengths):
    (attn_h, av, ma, pa), _ = run_hw(dict(
        source=source, main_embs=main_embs, pos_embs=pos_embs,
        W_main_in=W_main_in, W_pos_in=W_pos_in, W_out=W_out,
        memory_lengths=memory_lengths,
    ))
    return attn_h, av, ma, pa


# revision 59
# speedup vs baseline: 1.6562x; 1.6562x over previous
"""Trainium2 Bass kernel for DualAttention (general+pos scorer, renormalized).

Contract: kernel(**inputs) takes FULL unsharded numpy inputs and returns the
full outputs (attn_h, av, ma, pa) matching reference().

Strategy (data-parallel over batch, 8 batches per core on 8 cores):
  - q_b = W_main_in.T @ s_b computed on PE as a broadcast matmul (step-0
    free-dim lhsT replicates s_b across all 128 output partitions).
  - main_align = me_b @ q_b : fused multiply+accumulate on DVE
    (scalar_tensor_tensor with accum_out) while streaming me once (64MB).
  - pos_align = pe_b @ qp_b : DVE mult + 3D reduce over the pos stream.
  - softmax: DVE rowmax, GPSIMD partition_all_reduce (max/sum broadcast),
    ACT exp with accum_out.  av = exp(m+p - Mm - Mp)/Z (renorm identity).
  - c_t = sum_l avu[l]*me[l] on PE (K=128 l's, N=512, float32r = 1 cyc/row)
    from SBUF-resident me, using UNNORMALIZED weights so the matmuls don't
    wait for the global sum; c_t is scaled by 1/Z during PSUM evacuation.
  - attn_h = tanh(W_out @ [c_t; s]) with host-transposed W_out, batched over
    the core's 8 batches in two halves.
Host side does only sharding and pure relayouts (transposes, additive masks).
"""

import os
import sys

for _p in ("/opt/trn_rl_repo", "/root/.axon_site/_ro/trn_rl_repo"):
    if os.path.isdir(_p) and _p not in sys.path:
        sys.path.insert(0, _p)

import numpy as np

B, L, D, DP = 64, 4096, 512, 64
NCORES = 8
BL = B // NCORES          # local batches per core
P = 128                   # partitions
NT = L // P               # 32 l-tiles
NCH = 4                   # me chunks per batch
TCH = NT // NCH           # 8 l-tiles per me chunk
PCH = 8                   # pos chunks
PT = NT // PCH            # 4 l-tiles per pos chunk

CTX_DTYPE = os.environ.get("CTX_DTYPE", "float32r")  # float32 | float32r

_CACHE = {}


def _build():
    import concourse.bass as bass
    import concourse.bacc as bacc
    import concourse.tile as tile
    from concourse import mybir
    from concourse import bass_isa

    f32 = mybir.dt.float32
    ctx_dt = getattr(mybir.dt, CTX_DTYPE)

    nc = bacc.Bacc("TRN2", target_bir_lowering=False, debug=False)

    me_d = nc.dram_tensor("me", [L, BL, D], f32, kind="ExternalInput").ap()
    pos_d = nc.dram_tensor("pos", [L, BL, DP], f32, kind="ExternalInput").ap()
    srcT_d = nc.dram_tensor("srcT", [P, D // P, BL], f32, kind="ExternalInput").ap()
    wmain_d = nc.dram_tensor("wmain", [D, D], f32, kind="ExternalInput").ap()
    wpos_d = nc.dram_tensor("wpos", [D, DP], f32, kind="ExternalInput").ap()
    woutT_d = nc.dram_tensor("woutT", [2 * D, D], f32, kind="ExternalInput").ap()
    mask_d = nc.dram_tensor("mask2", [P, BL, 2, NT], f32, kind="ExternalInput").ap()

    oah_d = nc.dram_tensor("out_ah", [P, D // P, BL], f32, kind="ExternalOutput").ap()
    oma_d = nc.dram_tensor("out_ma", [P, BL, NT], f32, kind="ExternalOutput").ap()
    opa_d = nc.dram_tensor("out_pa", [P, BL, NT], f32, kind="ExternalOutput").ap()
    oav_d = nc.dram_tensor("out_av", [P, BL, NT], f32, kind="ExternalOutput").ap()

    AF = mybir.ActivationFunctionType
    AL = mybir.AluOpType
    AX = mybir.AxisListType

    # DRAM views: l = (c, t, p) row-major split
    me_r = me_d.rearrange("(c t p) b d -> c b p t d", p=P, t=TCH)
    pos_r = pos_d.rearrange("(c t p) b d -> c p t b d", p=P, t=PT)
    wm_r = wmain_d.rearrange("(h p) d -> h p d", p=P)
    wp_r = wpos_d.rearrange("(h p) d -> p h d", p=P)
    wo_r = woutT_d.rearrange("(h p) i -> p h i", p=P)

    with tile.TileContext(nc) as tc:
        with (
            tc.tile_pool(name="init", bufs=1) as init,
            tc.tile_pool(name="mep", bufs=7) as mep,
            tc.tile_pool(name="posp", bufs=3) as posp,
            tc.tile_pool(name="small", bufs=3) as small,
            tc.tile_pool(name="misc", bufs=2) as misc,
            tc.tile_pool(name="dump", bufs=3) as dump,
            tc.tile_pool(name="ps_tp", bufs=2, space="PSUM") as ps_tp,
            tc.tile_pool(name="ps_ctx", bufs=2, space="PSUM") as ps_ctx,
        ):
            # ---------- boot ----------
            st = init.tile([P, D // P, BL], f32, tag="st")
            nc.sync.dma_start(out=st, in_=srcT_d)
            # W_main in 4 separate chunk loads so the first query matmul can
            # start after 256KB instead of 1MB
            wmc = []
            for h in range(D // P):
                w1 = init.tile([P, D], f32, tag=f"wm{h}")
                nc.sync.dma_start(out=w1, in_=wm_r[h])
                wmc.append(w1)
            wp_t = init.tile([P, D // P, DP], f32, tag="wp")
            nc.sync.dma_start(out=wp_t, in_=wp_r)

            ones1 = init.tile([1, P], f32, tag="ones1")
            nc.vector.memset(ones1, 1.0)
            ident1 = init.tile([1, 1], f32, tag="ident1")
            nc.vector.memset(ident1, 1.0)

            QB = init.tile([P, BL, D], f32, tag="qb")
            QPB = init.tile([P, BL, DP], f32, tag="qpb")
            ALP = init.tile([P, BL, NT], f32, tag="alp")
            MAo = init.tile([P, BL, NT], f32, tag="mao")
            PAo = init.tile([P, BL, NT], f32, tag="pao")
            AVo = init.tile([P, BL, NT], f32, tag="avo")
            CCT = init.tile([P, D // P, BL], f32, tag="cct")
            AH = init.tile([P, D // P, BL], f32, tag="ah")

            with tc.tile_pool(name="ps_boot", bufs=2, space="PSUM") as psb:
                # warm the PE (HAM clock gate releases after ~3.4us of
                # sustained activity) with dep-free dummy matmuls
                wrm = psb.tile([1, P], f32, tag="bb")
                for _ in range(6):
                    nc.tensor.matmul(wrm, lhsT=ones1[:, 0:1], rhs=ones1,
                                     start=True, stop=True)

                # per-batch broadcast queries: lhsT = s_b column with a
                # step-0 free dim -> QB[p, d] = sum_e s_b[e] W[e, d] for all p
                for b in range(BL):
                    bb_p = psb.tile([P, D], f32, tag="bb")
                    for h in range(D // P):
                        col = st[:, h, b : b + 1]
                        lhsT = bass.AP(tensor=col.tensor, offset=col.offset,
                                       ap=[col.ap[0], [0, P]])
                        nc.tensor.matmul(
                            bb_p, lhsT=lhsT, rhs=wmc[h],
                            start=(h == 0), stop=(h == D // P - 1),
                        )
                    nc.scalar.copy(QB[:, b, :], bb_p)
                    bp_p = psb.tile([P, DP], f32, tag="bp")
                    for h in range(D // P):
                        col = st[:, h, b : b + 1]
                        lhsT = bass.AP(tensor=col.tensor, offset=col.offset,
                                       ap=[col.ap[0], [0, P]])
                        nc.tensor.matmul(
                            bp_p, lhsT=lhsT, rhs=wp_t[:, h, :],
                            start=(h == 0), stop=(h == D // P - 1),
                        )
                    nc.scalar.copy(QPB[:, b, :], bp_p)

            # hoist batch-0 me loads ahead of the pos stream
            b0_mes = []
            for c in range(NCH):
                met = mep.tile([P, TCH, D], ctx_dt, tag="me")
                nc.sync.dma_start(out=met, in_=me_r[c, 0].bitcast(ctx_dt))
                b0_mes.append(met)

            # ---------- pos aligns for all batches (stream pos once) -------
            for pc in range(PCH):
                pt = posp.tile([P, PT, BL, DP], f32, tag="pt")
                nc.sync.dma_start(out=pt, in_=pos_r[pc])
                for t in range(PT):
                    nc.vector.tensor_tensor(
                        out=pt[:, t, :, :], in0=pt[:, t, :, :], in1=QPB,
                        op=AL.mult,
                    )
                out_ap = ALP[:, :, pc * PT : (pc + 1) * PT].rearrange(
                    "p b t -> p t b"
                )
                nc.vector.tensor_reduce(out=out_ap, in_=pt, axis=AX.X, op=AL.add)

            # tail-only / later-needed loads, emitted after the hot boot path
            wt = init.tile([P, 2 * D // P, D], f32, tag="wt")
            nc.sync.dma_start(out=wt, in_=wo_r)
            mask_t = init.tile([P, BL, 2, NT], f32, tag="mask")
            nc.sync.dma_start(out=mask_t, in_=mask_d)

            # ---------- per-batch: main align, softmax, context ----------
            for b in range(BL):
                mes = []
                alm = small.tile([P, NT], f32, tag="alm")
                for c in range(NCH):
                    if b == 0:
                        met = b0_mes[c]
                    else:
                        met = mep.tile([P, TCH, D], ctx_dt, tag="me")
                        nc.sync.dma_start(out=met, in_=me_r[c, b].bitcast(ctx_dt))
                    mes.append(met)
                    for t in range(TCH):
                        dmp = dump.tile([P, D], f32, tag="dmp")
                        gt = c * TCH + t
                        nc.vector.scalar_tensor_tensor(
                            out=dmp, in0=met[:, t, :].bitcast(f32), scalar=1.0,
                            in1=QB[:, b, :], op0=AL.mult, op1=AL.mult,
                            accum_out=alm[:, gt : gt + 1],
                        )

                # masked scores: am[:,0,:]=main, am[:,1,:]=pos
                am = small.tile([P, 2, NT], f32, tag="am")
                nc.vector.tensor_tensor(out=am[:, 0, :], in0=alm,
                                        in1=mask_t[:, b, 0, :], op=AL.add)
                nc.vector.tensor_tensor(out=am[:, 1, :], in0=ALP[:, b, :],
                                        in1=mask_t[:, b, 1, :], op=AL.add)
                m2 = small.tile([P, 2], f32, tag="m2")
                nc.vector.tensor_reduce(out=m2, in_=am, axis=AX.X, op=AL.max)
                g2 = small.tile([P, 2], f32, tag="g2")
                nc.gpsimd.partition_all_reduce(
                    g2, m2, channels=P, reduce_op=bass_isa.ReduceOp.max
                )
                ng2 = small.tile([P, 2], f32, tag="ng2")
                nc.scalar.mul(ng2, g2, -1.0)

                ex = small.tile([P, 2, NT], f32, tag="ex")
                s3 = small.tile([P, 3], f32, tag="s3")
                nc.scalar.activation(
                    out=ex[:, 0, :], in_=am[:, 0, :], func=AF.Exp,
                    bias=ng2[:, 0:1], scale=1.0, accum_out=s3[:, 0:1],
                )
                nc.scalar.activation(
                    out=ex[:, 1, :], in_=am[:, 1, :], func=AF.Exp,
                    bias=ng2[:, 1:2], scale=1.0, accum_out=s3[:, 1:2],
                )
                avu = small.tile([P, NT], f32, tag="avu")
                nc.vector.scalar_tensor_tensor(
                    out=avu, in0=ex[:, 0, :], scalar=1.0, in1=ex[:, 1, :],
                    op0=AL.mult, op1=AL.mult, accum_out=s3[:, 2:3],
                )
                # context weights: unnormalized avu rounded to the context
                # dtype; normalization folded into the PSUM evacuation below
                avr = small.tile([P, NT], ctx_dt, tag="avr")
                nc.scalar.copy(avr, avu)

                z3 = small.tile([P, 3], f32, tag="z3")
                nc.gpsimd.partition_all_reduce(
                    z3, s3, channels=P, reduce_op=bass_isa.ReduceOp.add
                )
                r3 = small.tile([P, 3], f32, tag="r3")
                nc.vector.reciprocal(r3, z3)

                nc.scalar.activation(out=MAo[:, b, :], in_=ex[:, 0, :],
                                     func=AF.Copy, scale=r3[:, 0:1])
                nc.sync.dma_start(out=oma_d[:, b, :], in_=MAo[:, b, :])
                nc.scalar.activation(out=PAo[:, b, :], in_=ex[:, 1, :],
                                     func=AF.Copy, scale=r3[:, 1:2])
                nc.sync.dma_start(out=opa_d[:, b, :], in_=PAo[:, b, :])
                nc.scalar.activation(out=AVo[:, b, :], in_=avu,
                                     func=AF.Copy, scale=r3[:, 2:3])
                nc.sync.dma_start(out=oav_d[:, b, :], in_=AVo[:, b, :])

                # context: c_t = (1/Z) * sum_l avu[l] * me[l, :]
                ctx_p = ps_ctx.tile([1, D], f32, tag="ctx")
                for c in range(NCH):
                    for t in range(TCH):
                        gt = c * TCH + t
                        nc.tensor.matmul(
                            ctx_p,
                            lhsT=avr[:, gt : gt + 1],
                            rhs=mes[c][:, t, :],
                            start=(gt == 0), stop=(gt == NT - 1),
                        )
                cts = misc.tile([1, D], f32, tag="cts")
                nc.scalar.activation(out=cts, in_=ctx_p, func=AF.Copy,
                                     scale=r3[0:1, 2:3])
                for k in range(D // P):
                    tp_p = ps_tp.tile([P, 1], f32, tag="tp")
                    nc.tensor.transpose(
                        tp_p, cts[0:1, k * P : (k + 1) * P], ident1
                    )
                    nc.scalar.copy(CCT[:, k, b : b + 1], tp_p)

            # ---------- tail: attn_h = tanh(W_out @ [c_t; s]) ----------
            with tc.tile_pool(name="ps_ah", bufs=3, space="PSUM") as psah:
                for lo, hi in ((0, BL // 2), (BL // 2, BL)):
                    for ic in range(D // P):
                        ah_p = psah.tile([P, hi - lo], f32, tag="ahp")
                        for jc in range(2 * D // P):
                            rhs = (CCT[:, jc, lo:hi] if jc < D // P
                                   else st[:, jc - D // P, lo:hi])
                            nc.tensor.matmul(
                                ah_p, lhsT=wt[:, jc, ic * P : (ic + 1) * P],
                                rhs=rhs,
                                start=(jc == 0), stop=(jc == 2 * D // P - 1),
                            )
                        nc.scalar.activation(out=AH[:, ic, lo:hi], in_=ah_p,
                                             func=AF.Tanh)

            nc.sync.dma_start(out=oah_d, in_=AH)

    nc.compile()
    return nc


def _get_nc():
    if "nc" not in _CACHE:
        _CACHE["nc"] = _build()
    return _CACHE["nc"]


def _make_in_maps(source, main_embs, pos_embs, W_main_in, W_pos_in, W_out,
                  memory_lengths):
    source = np.asarray(source, np.float32)
    main_embs = np.asarray(main_embs, np.float32)
    pos_embs = np.asarray(pos_embs, np.float32)
    W_main_in = np.ascontiguousarray(np.asarray(W_main_in, np.float32))
    W_pos_in = np.ascontiguousarray(np.asarray(W_pos_in, np.float32))
    woutT = np.ascontiguousarray(np.asarray(W_out, np.float32).T)
    lens = np.asarray(memory_lengths)

    maskadd = np.where(
        np.arange(L)[None, :] < np.asarray(lens, np.int64)[:, None], 0.0, -1e30
    ).astype(np.float32)                      # (B, L)
    m = maskadd.reshape(B, NT, P).transpose(2, 0, 1)  # (P, B, NT)
    mask2 = np.ascontiguousarray(np.stack([m, m], axis=2))  # (P, B, 2, NT)

    in_maps = []
    for c in range(NCORES):
        sl = slice(c * BL, (c + 1) * BL)
        srcT = np.ascontiguousarray(
            source[sl].T.reshape(D // P, P, BL).transpose(1, 0, 2)
        )  # (P, 4, BL): srcT[p,h,b] = source[c*BL+b, h*P+p]
        in_maps.append({
            "me": np.ascontiguousarray(main_embs[:, sl, :]),
            "pos": np.ascontiguousarray(pos_embs[:, sl, :]),
            "srcT": srcT,
            "wmain": W_main_in,
            "wpos": W_pos_in,
            "woutT": woutT,
            "mask2": np.ascontiguousarray(mask2[:, sl]),
        })
    return in_maps


def _assemble(results):
    ah_rows, ma_rows, pa_rows, av_rows = [], [], [], []
    for r in results:
        # out_ah (P, 4, BL): attn[b, h*P+p] = out_ah[p, h, b]
        ah_rows.append(np.ascontiguousarray(
            np.asarray(r["out_ah"]).transpose(2, 1, 0).reshape(BL, D)))
        for rows, key in ((ma_rows, "out_ma"), (pa_rows, "out_pa"),
                          (av_rows, "out_av")):
            # (P, BL, NT): x[b, t*P+p] = arr[p, b, t]
            rows.append(np.ascontiguousarray(
                np.asarray(r[key]).transpose(1, 2, 0).reshape(BL, L)))
    attn_h = np.concatenate(ah_rows, 0)
    ma = np.concatenate(ma_rows, 0)
    pa = np.concatenate(pa_rows, 0)
    av = np.concatenate(av_rows, 0)
    return attn_h, av, ma, pa


def run_hw(inputs, trace=False, **kw):
    from concourse import bass_utils
    nc = _get_nc()
    in_maps = _make_in_maps(**inputs)
    res = bass_utils.run_bass_kernel_spmd(
        nc, in_maps, core_ids=list(range(NCORES)), trace=trace, **kw
    )
    return _assemble(res.results), res


def kernel(source, main_embs, pos_embs, W_main_in, W_pos_in, W_out,
           memory_lengths):
    (attn_h, av, ma, pa), _ = run_hw(dict(
        source=source, main_embs=main_embs, pos_embs=pos_embs,
        W_main_in=W_main_in, W_pos_in=W_pos_in, W_out=W_out,
        memory_lengths=memory_lengths,
    ))
    return attn_h, av, ma, pa


# revision 60
# speedup vs baseline: 1.7525x; 1.0581x over previous
"""Trainium2 Bass kernel for DualAttention (general+pos scorer, renormalized).

Contract: kernel(**inputs) takes FULL unsharded numpy inputs and returns the
full outputs (attn_h, av, ma, pa) matching reference().

Strategy (data-parallel over batch, 8 batches per core on 8 cores):
  - q_b = W_main_in.T @ s_b computed on PE as a broadcast matmul (step-0
    free-dim lhsT replicates s_b across all 128 output partitions).
  - main_align = me_b @ q_b : fused multiply+accumulate on DVE
    (scalar_tensor_tensor with accum_out) while streaming me once (64MB).
  - pos_align = pe_b @ qp_b : DVE mult + 3D reduce over the pos stream.
  - softmax: DVE rowmax, GPSIMD partition_all_reduce (max/sum broadcast),
    ACT exp with accum_out.  av = exp(m+p - Mm - Mp)/Z (renorm identity).
  - c_t = sum_l avu[l]*me[l] on PE (K=128 l's, N=512, float32r = 1 cyc/row)
    from SBUF-resident me, using UNNORMALIZED weights so the matmuls don't
    wait for the global sum; c_t is scaled by 1/Z during PSUM evacuation.
  - attn_h = tanh(W_out @ [c_t; s]) with host-transposed W_out, batched over
    the core's 8 batches in two halves.
Host side does only sharding and pure relayouts (transposes, additive masks).
"""

import os
import sys

for _p in ("/opt/trn_rl_repo", "/root/.axon_site/_ro/trn_rl_repo"):
    if os.path.isdir(_p) and _p not in sys.path:
        sys.path.insert(0, _p)

import numpy as np

B, L, D, DP = 64, 4096, 512, 64
NCORES = 8
BL = B // NCORES          # local batches per core
P = 128                   # partitions
NT = L // P               # 32 l-tiles
NCH = 4                   # me chunks per batch
TCH = NT // NCH           # 8 l-tiles per me chunk
PCH = 8                   # pos chunks
PT = NT // PCH            # 4 l-tiles per pos chunk

CTX_DTYPE = os.environ.get("CTX_DTYPE", "float32r")  # float32 | float32r

_CACHE = {}


def _build():
    import concourse.bass as bass
    import concourse.bacc as bacc
    import concourse.tile as tile
    from concourse import mybir
    from concourse import bass_isa

    f32 = mybir.dt.float32
    ctx_dt = getattr(mybir.dt, CTX_DTYPE)

    nc = bacc.Bacc("TRN2", target_bir_lowering=False, debug=False)

    me_d = nc.dram_tensor("me", [L, BL, D], f32, kind="ExternalInput").ap()
    pos_d = nc.dram_tensor("pos", [L, BL, DP], f32, kind="ExternalInput").ap()
    srcT_d = nc.dram_tensor("srcT", [P, D // P, BL], f32, kind="ExternalInput").ap()
    wmain_d = nc.dram_tensor("wmain", [D, D], f32, kind="ExternalInput").ap()
    wpos_d = nc.dram_tensor("wpos", [D, DP], f32, kind="ExternalInput").ap()
    woutT_d = nc.dram_tensor("woutT", [2 * D, D], f32, kind="ExternalInput").ap()
    mask_d = nc.dram_tensor("mask2", [P, BL, 2, NT], f32, kind="ExternalInput").ap()

    oah_d = nc.dram_tensor("out_ah", [P, D // P, BL], f32, kind="ExternalOutput").ap()
    oma_d = nc.dram_tensor("out_ma", [P, BL, NT], f32, kind="ExternalOutput").ap()
    opa_d = nc.dram_tensor("out_pa", [P, BL, NT], f32, kind="ExternalOutput").ap()
    oav_d = nc.dram_tensor("out_av", [P, BL, NT], f32, kind="ExternalOutput").ap()

    AF = mybir.ActivationFunctionType
    AL = mybir.AluOpType
    AX = mybir.AxisListType

    # DRAM views: l = (c, t, p) row-major split
    me_r = me_d.rearrange("(c t p) b d -> c b p t d", p=P, t=TCH)
    pos_r = pos_d.rearrange("(c t p) b d -> c p t b d", p=P, t=PT)
    wm_r = wmain_d.rearrange("(h p) d -> h p d", p=P)
    wp_r = wpos_d.rearrange("(h p) d -> p h d", p=P)
    wo_r = woutT_d.rearrange("(h p) i -> p h i", p=P)

    with tile.TileContext(nc) as tc:
        with (
            tc.tile_pool(name="init", bufs=1) as init,
            tc.tile_pool(name="mep", bufs=7) as mep,
            tc.tile_pool(name="posp", bufs=3) as posp,
            tc.tile_pool(name="small", bufs=3) as small,
            tc.tile_pool(name="misc", bufs=2) as misc,
            tc.tile_pool(name="dump", bufs=3) as dump,
            tc.tile_pool(name="ps_tp", bufs=2, space="PSUM") as ps_tp,
            tc.tile_pool(name="ps_ctx", bufs=2, space="PSUM") as ps_ctx,
        ):
            # ---------- boot ----------
            st = init.tile([P, D // P, BL], f32, tag="st")
            nc.sync.dma_start(out=st, in_=srcT_d)
            # W_main in 4 separate chunk loads so the first query matmul can
            # start after 256KB instead of 1MB
            wmc = []
            for h in range(D // P):
                w1 = init.tile([P, D], f32, tag=f"wm{h}")
                nc.sync.dma_start(out=w1, in_=wm_r[h])
                wmc.append(w1)
            wp_t = init.tile([P, D // P, DP], f32, tag="wp")
            nc.sync.dma_start(out=wp_t, in_=wp_r)

            ones1 = init.tile([1, P], f32, tag="ones1")
            nc.vector.memset(ones1, 1.0)
            ident1 = init.tile([1, 1], f32, tag="ident1")
            nc.vector.memset(ident1, 1.0)

            QB = init.tile([P, BL, D], f32, tag="qb")
            QPB = init.tile([P, BL, DP], f32, tag="qpb")
            ALP = init.tile([P, BL, NT], f32, tag="alp")
            MAo = init.tile([P, BL, NT], f32, tag="mao")
            PAo = init.tile([P, BL, NT], f32, tag="pao")
            AVo = init.tile([P, BL, NT], f32, tag="avo")
            CCT = init.tile([P, D // P, BL], f32, tag="cct")
            AH = init.tile([P, D // P, BL], f32, tag="ah")

            with tc.tile_pool(name="ps_boot", bufs=2, space="PSUM") as psb:
                # warm the PE (HAM clock gate releases after ~3.4us of
                # sustained activity) with dep-free dummy matmuls
                wrm = psb.tile([1, P], f32, tag="bb")
                for _ in range(6):
                    nc.tensor.matmul(wrm, lhsT=ones1[:, 0:1], rhs=ones1,
                                     start=True, stop=True)

                # per-batch broadcast queries: lhsT = s_b column with a
                # step-0 free dim -> QB[p, d] = sum_e s_b[e] W[e, d] for all p
                for b in range(BL):
                    bb_p = psb.tile([P, D], f32, tag="bb")
                    for h in range(D // P):
                        col = st[:, h, b : b + 1]
                        lhsT = bass.AP(tensor=col.tensor, offset=col.offset,
                                       ap=[col.ap[0], [0, P]])
                        nc.tensor.matmul(
                            bb_p, lhsT=lhsT, rhs=wmc[h],
                            start=(h == 0), stop=(h == D // P - 1),
                        )
                    nc.scalar.copy(QB[:, b, :], bb_p)
                    bp_p = psb.tile([P, DP], f32, tag="bp")
                    for h in range(D // P):
                        col = st[:, h, b : b + 1]
                        lhsT = bass.AP(tensor=col.tensor, offset=col.offset,
                                       ap=[col.ap[0], [0, P]])
                        nc.tensor.matmul(
                            bp_p, lhsT=lhsT, rhs=wp_t[:, h, :],
                            start=(h == 0), stop=(h == D // P - 1),
                        )
                    nc.scalar.copy(QPB[:, b, :], bp_p)

            # hoist batch-0 me loads ahead of the pos stream; chunk 0 is
            # loaded per-tile so the very first align op starts ~5us earlier
            b0_mes = []
            for c in range(NCH):
                met = mep.tile([P, TCH, D], ctx_dt, tag="me")
                if c == 0:
                    for t in range(TCH):
                        nc.sync.dma_start(
                            out=met[:, t, :],
                            in_=me_r[c, 0, :, t, :].bitcast(ctx_dt))
                else:
                    nc.sync.dma_start(out=met, in_=me_r[c, 0].bitcast(ctx_dt))
                b0_mes.append(met)

            # ---------- pos aligns for all batches (stream pos once) -------
            for pc in range(PCH):
                pt = posp.tile([P, PT, BL, DP], f32, tag="pt")
                nc.sync.dma_start(out=pt, in_=pos_r[pc])
                for t in range(PT):
                    nc.vector.tensor_tensor(
                        out=pt[:, t, :, :], in0=pt[:, t, :, :], in1=QPB,
                        op=AL.mult,
                    )
                out_ap = ALP[:, :, pc * PT : (pc + 1) * PT].rearrange(
                    "p b t -> p t b"
                )
                nc.vector.tensor_reduce(out=out_ap, in_=pt, axis=AX.X, op=AL.add)

            # tail-only / later-needed loads, emitted after the hot boot path
            wt = init.tile([P, 2 * D // P, D], f32, tag="wt")
            nc.sync.dma_start(out=wt, in_=wo_r)
            mask_t = init.tile([P, BL, 2, NT], f32, tag="mask")
            nc.sync.dma_start(out=mask_t, in_=mask_d)

            # ---------- per-batch: main align, softmax, context ----------
            for b in range(BL):
                mes = []
                alm = small.tile([P, NT], f32, tag="alm")
                for c in range(NCH):
                    if b == 0:
                        met = b0_mes[c]
                    else:
                        met = mep.tile([P, TCH, D], ctx_dt, tag="me")
                        nc.sync.dma_start(out=met, in_=me_r[c, b].bitcast(ctx_dt))
                    mes.append(met)
                    for t in range(TCH):
                        dmp = dump.tile([P, D], f32, tag="dmp")
                        gt = c * TCH + t
                        nc.vector.scalar_tensor_tensor(
                            out=dmp, in0=met[:, t, :].bitcast(f32), scalar=1.0,
                            in1=QB[:, b, :], op0=AL.mult, op1=AL.mult,
                            accum_out=alm[:, gt : gt + 1],
                        )

                # masked scores: am[:,0,:]=main, am[:,1,:]=pos
                am = small.tile([P, 2, NT], f32, tag="am")
                nc.vector.tensor_tensor(out=am[:, 0, :], in0=alm,
                                        in1=mask_t[:, b, 0, :], op=AL.add)
                nc.vector.tensor_tensor(out=am[:, 1, :], in0=ALP[:, b, :],
                                        in1=mask_t[:, b, 1, :], op=AL.add)
                m2 = small.tile([P, 2], f32, tag="m2")
                nc.vector.tensor_reduce(out=m2, in_=am, axis=AX.X, op=AL.max)
                g2 = small.tile([P, 2], f32, tag="g2")
                nc.gpsimd.partition_all_reduce(
                    g2, m2, channels=P, reduce_op=bass_isa.ReduceOp.max
                )
                ng2 = small.tile([P, 2], f32, tag="ng2")
                nc.scalar.mul(ng2, g2, -1.0)

                ex = small.tile([P, 2, NT], f32, tag="ex")
                s3 = small.tile([P, 3], f32, tag="s3")
                nc.scalar.activation(
                    out=ex[:, 0, :], in_=am[:, 0, :], func=AF.Exp,
                    bias=ng2[:, 0:1], scale=1.0, accum_out=s3[:, 0:1],
                )
                nc.scalar.activation(
                    out=ex[:, 1, :], in_=am[:, 1, :], func=AF.Exp,
                    bias=ng2[:, 1:2], scale=1.0, accum_out=s3[:, 1:2],
                )
                avu = small.tile([P, NT], f32, tag="avu")
                nc.vector.scalar_tensor_tensor(
                    out=avu, in0=ex[:, 0, :], scalar=1.0, in1=ex[:, 1, :],
                    op0=AL.mult, op1=AL.mult, accum_out=s3[:, 2:3],
                )
                # context weights: unnormalized avu rounded to the context
                # dtype; normalization folded into the PSUM evacuation below
                avr = small.tile([P, NT], ctx_dt, tag="avr")
                nc.scalar.copy(avr, avu)

                z3 = small.tile([P, 3], f32, tag="z3")
                nc.gpsimd.partition_all_reduce(
                    z3, s3, channels=P, reduce_op=bass_isa.ReduceOp.add
                )
                r3 = small.tile([P, 3], f32, tag="r3")
                nc.vector.reciprocal(r3, z3)

                nc.scalar.activation(out=MAo[:, b, :], in_=ex[:, 0, :],
                                     func=AF.Copy, scale=r3[:, 0:1])
                nc.sync.dma_start(out=oma_d[:, b, :], in_=MAo[:, b, :])
                nc.scalar.activation(out=PAo[:, b, :], in_=ex[:, 1, :],
                                     func=AF.Copy, scale=r3[:, 1:2])
                nc.sync.dma_start(out=opa_d[:, b, :], in_=PAo[:, b, :])
                nc.scalar.activation(out=AVo[:, b, :], in_=avu,
                                     func=AF.Copy, scale=r3[:, 2:3])
                nc.sync.dma_start(out=oav_d[:, b, :], in_=AVo[:, b, :])

                # context: c_t = (1/Z) * sum_l avu[l] * me[l, :]
                ctx_p = ps_ctx.tile([1, D], f32, tag="ctx")
                for c in range(NCH):
                    for t in range(TCH):
                        gt = c * TCH + t
                        nc.tensor.matmul(
                            ctx_p,
                            lhsT=avr[:, gt : gt + 1],
                            rhs=mes[c][:, t, :],
                            start=(gt == 0), stop=(gt == NT - 1),
                        )
                cts = misc.tile([1, D], f32, tag="cts")
                nc.scalar.activation(out=cts, in_=ctx_p, func=AF.Copy,
                                     scale=r3[0:1, 2:3])
                for k in range(D // P):
                    tp_p = ps_tp.tile([P, 1], f32, tag="tp")
                    nc.tensor.transpose(
                        tp_p, cts[0:1, k * P : (k + 1) * P], ident1
                    )
                    nc.scalar.copy(CCT[:, k, b : b + 1], tp_p)

            # ---------- tail: attn_h = tanh(W_out @ [c_t; s]) ----------
            with tc.tile_pool(name="ps_ah", bufs=3, space="PSUM") as psah:
                for lo, hi in ((0, 6), (6, BL)):
                    for ic in range(D // P):
                        ah_p = psah.tile([P, hi - lo], f32, tag="ahp")
                        for jc in range(2 * D // P):
                            rhs = (CCT[:, jc, lo:hi] if jc < D // P
                                   else st[:, jc - D // P, lo:hi])
                            nc.tensor.matmul(
                                ah_p, lhsT=wt[:, jc, ic * P : (ic + 1) * P],
                                rhs=rhs,
                                start=(jc == 0), stop=(jc == 2 * D // P - 1),
                            )
                        nc.scalar.activation(out=AH[:, ic, lo:hi], in_=ah_p,
                                             func=AF.Tanh)

            nc.sync.dma_start(out=oah_d, in_=AH)

    nc.compile()
    return nc


def _get_nc():
    if "nc" not in _CACHE:
        _CACHE["nc"] = _build()
    return _CACHE["nc"]


def _make_in_maps(source, main_embs, pos_embs, W_main_in, W_pos_in, W_out,
                  memory_lengths):
    source = np.asarray(source, np.float32)
    main_embs = np.asarray(main_embs, np.float32)
    pos_embs = np.asarray(pos_embs, np.float32)
    W_main_in = np.ascontiguousarray(np.asarray(W_main_in, np.float32))
    W_pos_in = np.ascontiguousarray(np.asarray(W_pos_in, np.float32))
    woutT = np.ascontiguousarray(np.asarray(W_out, np.float32).T)
    lens = np.asarray(memory_lengths)

    maskadd = np.where(
        np.arange(L)[None, :] < np.asarray(lens, np.int64)[:, None], 0.0, -1e30
    ).astype(np.float32)                      # (B, L)
    m = maskadd.reshape(B, NT, P).transpose(2, 0, 1)  # (P, B, NT)
    mask2 = np.ascontiguousarray(np.stack([m, m], axis=2))  # (P, B, 2, NT)

    in_maps = []
    for c in range(NCORES):
        sl = slice(c * BL, (c + 1) * BL)
        srcT = np.ascontiguousarray(
            source[sl].T.reshape(D // P, P, BL).transpose(1, 0, 2)
        )  # (P, 4, BL): srcT[p,h,b] = source[c*BL+b, h*P+p]
        in_maps.append({
            "me": np.ascontiguousarray(main_embs[:, sl, :]),
            "pos": np.ascontiguousarray(pos_embs[:, sl, :]),
            "srcT": srcT,
            "wmain": W_main_in,
            "wpos": W_pos_in,
            "woutT": woutT,
            "mask2": np.ascontiguousarray(mask2[:, sl]),
        })
    return in_maps


def _assemble(results):
    ah_rows, ma_rows, pa_rows, av_rows = [], [], [], []
    for r in results:
        # out_ah (P, 4, BL): attn[b, h*P+p] = out_ah[p, h, b]
        ah_rows.append(np.ascontiguousarray(
            np.asarray(r["out_ah"]).transpose(2, 1, 0).reshape(BL, D)))
        for rows, key in ((ma_rows, "out_ma"), (pa_rows, "out_pa"),
                          (av_rows, "out_av")):
            # (P, BL, NT): x[b, t*P+p] = arr[p, b, t]
            rows.append(np.ascontiguousarray(
                np.asarray(r[key]).transpose(1, 2, 0).reshape(BL, L)))
    attn_h = np.concatenate(ah_rows, 0)
    ma = np.concatenate(ma_rows, 0)
    pa = np.concatenate(pa_rows, 0)
    av = np.concatenate(av_rows, 0)
    return attn_h, av, ma, pa


def run_hw(inputs, trace=False, **kw):
    from concourse import bass_utils
    nc = _get_nc()
    in_maps = _make_in_maps(**inputs)
    res = bass_utils.run_bass_kernel_spmd(
        nc, in_maps, core_ids=list(range(NCORES)), trace=trace, **kw
    )
    return _assemble(res.results), res


def kernel(source, main_embs, pos_embs, W_main_in, W_pos_in, W_out,
           memory_lengths):
    (attn_h, av, ma, pa), _ = run_hw(dict(
        source=source, main_embs=main_embs, pos_embs=pos_embs,
        W_main_in=W_main_in, W_pos_in=W_pos_in, W_out=W_out,
        memory_lengths=memory_lengths,
    ))
    return attn_h, av, ma, pa


# revision 61
# speedup vs baseline: 1.7721x; 1.0112x over previous
"""Trainium2 Bass kernel for DualAttention (general+pos scorer, renormalized).

Contract: kernel(**inputs) takes FULL unsharded numpy inputs and returns the
full outputs (attn_h, av, ma, pa) matching reference().

Strategy (data-parallel over batch, 8 batches per core on 8 cores):
  - q_b = W_main_in.T @ s_b computed on PE as a broadcast matmul (step-0
    free-dim lhsT replicates s_b across all 128 output partitions).
  - main_align = me_b @ q_b : fused multiply+accumulate on DVE
    (scalar_tensor_tensor with accum_out) while streaming me once (64MB).
  - pos_align = pe_b @ qp_b : DVE mult + 3D reduce over the pos stream.
  - softmax: DVE rowmax, GPSIMD partition_all_reduce (max/sum broadcast),
    ACT exp with accum_out.  av = exp(m+p - Mm - Mp)/Z (renorm identity).
  - c_t = sum_l avu[l]*me[l] on PE (K=128 l's, N=512, float32r = 1 cyc/row)
    from SBUF-resident me, using UNNORMALIZED weights so the matmuls don't
    wait for the global sum; c_t is scaled by 1/Z during PSUM evacuation.
  - attn_h = tanh(W_out @ [c_t; s]) with host-transposed W_out, batched over
    the core's 8 batches in two halves.
Host side does only sharding and pure relayouts (transposes, additive masks).
"""

import os
import sys

for _p in ("/opt/trn_rl_repo", "/root/.axon_site/_ro/trn_rl_repo"):
    if os.path.isdir(_p) and _p not in sys.path:
        sys.path.insert(0, _p)

import numpy as np

B, L, D, DP = 64, 4096, 512, 64
NCORES = 8
BL = B // NCORES          # local batches per core
P = 128                   # partitions
NT = L // P               # 32 l-tiles
NCH = 4                   # me chunks per batch
TCH = NT // NCH           # 8 l-tiles per me chunk
PCH = 8                   # pos chunks
PT = NT // PCH            # 4 l-tiles per pos chunk

CTX_DTYPE = os.environ.get("CTX_DTYPE", "float32r")  # float32 | float32r

_CACHE = {}


def _build():
    import concourse.bass as bass
    import concourse.bacc as bacc
    import concourse.tile as tile
    from concourse import mybir
    from concourse import bass_isa

    f32 = mybir.dt.float32
    ctx_dt = getattr(mybir.dt, CTX_DTYPE)

    nc = bacc.Bacc("TRN2", target_bir_lowering=False, debug=False)

    me_d = nc.dram_tensor("me", [L, BL, D], f32, kind="ExternalInput").ap()
    pos_d = nc.dram_tensor("pos", [L, BL, DP], f32, kind="ExternalInput").ap()
    srcT_d = nc.dram_tensor("srcT", [P, D // P, BL], f32, kind="ExternalInput").ap()
    wmain_d = nc.dram_tensor("wmain", [D, D], f32, kind="ExternalInput").ap()
    wpos_d = nc.dram_tensor("wpos", [D, DP], f32, kind="ExternalInput").ap()
    woutT_d = nc.dram_tensor("woutT", [2 * D, D], f32, kind="ExternalInput").ap()
    mask_d = nc.dram_tensor("mask2", [P, BL, 2, NT], f32, kind="ExternalInput").ap()

    oah_d = nc.dram_tensor("out_ah", [BL, D], f32, kind="ExternalOutput").ap()
    oma_d = nc.dram_tensor("out_ma", [P, BL, NT], f32, kind="ExternalOutput").ap()
    opa_d = nc.dram_tensor("out_pa", [P, BL, NT], f32, kind="ExternalOutput").ap()
    oav_d = nc.dram_tensor("out_av", [P, BL, NT], f32, kind="ExternalOutput").ap()

    AF = mybir.ActivationFunctionType
    AL = mybir.AluOpType
    AX = mybir.AxisListType

    # DRAM views: l = (c, t, p) row-major split
    me_r = me_d.rearrange("(c t p) b d -> c b p t d", p=P, t=TCH)
    pos_r = pos_d.rearrange("(c t p) b d -> c p t b d", p=P, t=PT)
    wm_r = wmain_d.rearrange("(h p) d -> h p d", p=P)
    wp_r = wpos_d.rearrange("(h p) d -> p h d", p=P)
    wo_r = woutT_d.rearrange("(h p) i -> p h i", p=P)

    with tile.TileContext(nc) as tc:
        with (
            tc.tile_pool(name="init", bufs=1) as init,
            tc.tile_pool(name="mep", bufs=7) as mep,
            tc.tile_pool(name="posp", bufs=3) as posp,
            tc.tile_pool(name="small", bufs=3) as small,
            tc.tile_pool(name="misc", bufs=2) as misc,
            tc.tile_pool(name="dump", bufs=3) as dump,
            tc.tile_pool(name="ps_tp", bufs=2, space="PSUM") as ps_tp,
            tc.tile_pool(name="ps_ctx", bufs=2, space="PSUM") as ps_ctx,
        ):
            # ---------- boot ----------
            st = init.tile([P, D // P, BL], f32, tag="st")
            nc.sync.dma_start(out=st, in_=srcT_d)
            # W_main in 4 separate chunk loads so the first query matmul can
            # start after 256KB instead of 1MB
            wmc = []
            for h in range(D // P):
                w1 = init.tile([P, D], f32, tag=f"wm{h}")
                nc.sync.dma_start(out=w1, in_=wm_r[h])
                wmc.append(w1)
            wp_t = init.tile([P, D // P, DP], f32, tag="wp")
            nc.sync.dma_start(out=wp_t, in_=wp_r)

            ones1 = init.tile([1, P], f32, tag="ones1")
            nc.vector.memset(ones1, 1.0)
            ident1 = init.tile([1, 1], f32, tag="ident1")
            nc.vector.memset(ident1, 1.0)

            QB = init.tile([P, BL, D], f32, tag="qb")
            QPB = init.tile([P, BL, DP], f32, tag="qpb")
            ALP = init.tile([P, BL, NT], f32, tag="alp")
            MAo = init.tile([P, BL, NT], f32, tag="mao")
            PAo = init.tile([P, BL, NT], f32, tag="pao")
            AVo = init.tile([P, BL, NT], f32, tag="avo")
            CCT = init.tile([P, D // P, BL], ctx_dt, tag="cct")
            stR = init.tile([P, D // P, BL], ctx_dt, tag="str")
            nc.sync.dma_start(out=stR, in_=srcT_d.bitcast(ctx_dt))

            with tc.tile_pool(name="ps_boot", bufs=2, space="PSUM") as psb:
                # warm the PE (HAM clock gate releases after ~3.4us of
                # sustained activity) with dep-free dummy matmuls
                wrm = psb.tile([1, P], f32, tag="bb")
                for _ in range(6):
                    nc.tensor.matmul(wrm, lhsT=ones1[:, 0:1], rhs=ones1,
                                     start=True, stop=True)

                # per-batch broadcast queries: lhsT = s_b column with a
                # step-0 free dim -> QB[p, d] = sum_e s_b[e] W[e, d] for all p
                for b in range(BL):
                    bb_p = psb.tile([P, D], f32, tag="bb")
                    for h in range(D // P):
                        col = st[:, h, b : b + 1]
                        lhsT = bass.AP(tensor=col.tensor, offset=col.offset,
                                       ap=[col.ap[0], [0, P]])
                        nc.tensor.matmul(
                            bb_p, lhsT=lhsT, rhs=wmc[h],
                            start=(h == 0), stop=(h == D // P - 1),
                        )
                    nc.scalar.copy(QB[:, b, :], bb_p)
                    bp_p = psb.tile([P, DP], f32, tag="bp")
                    for h in range(D // P):
                        col = st[:, h, b : b + 1]
                        lhsT = bass.AP(tensor=col.tensor, offset=col.offset,
                                       ap=[col.ap[0], [0, P]])
                        nc.tensor.matmul(
                            bp_p, lhsT=lhsT, rhs=wp_t[:, h, :],
                            start=(h == 0), stop=(h == D // P - 1),
                        )
                    nc.scalar.copy(QPB[:, b, :], bp_p)

            # hoist batch-0 me loads ahead of the pos stream; chunk 0 is
            # loaded per-tile so the very first align op starts ~5us earlier
            b0_mes = []
            for c in range(NCH):
                met = mep.tile([P, TCH, D], ctx_dt, tag="me")
                if c == 0:
                    for t in range(TCH):
                        nc.sync.dma_start(
                            out=met[:, t, :],
                            in_=me_r[c, 0, :, t, :].bitcast(ctx_dt))
                else:
                    nc.sync.dma_start(out=met, in_=me_r[c, 0].bitcast(ctx_dt))
                b0_mes.append(met)

            # ---------- pos aligns for all batches (stream pos once) -------
            for pc in range(PCH):
                pt = posp.tile([P, PT, BL, DP], f32, tag="pt")
                nc.sync.dma_start(out=pt, in_=pos_r[pc])
                for t in range(PT):
                    nc.vector.tensor_tensor(
                        out=pt[:, t, :, :], in0=pt[:, t, :, :], in1=QPB,
                        op=AL.mult,
                    )
                out_ap = ALP[:, :, pc * PT : (pc + 1) * PT].rearrange(
                    "p b t -> p t b"
                )
                nc.vector.tensor_reduce(out=out_ap, in_=pt, axis=AX.X, op=AL.add)

            # tail-only / later-needed loads, emitted after the hot boot path
            wt = init.tile([P, 2 * D // P, D], ctx_dt, tag="wt")
            nc.sync.dma_start(out=wt, in_=wo_r.bitcast(ctx_dt))
            mask_t = init.tile([P, BL, 2, NT], f32, tag="mask")
            nc.sync.dma_start(out=mask_t, in_=mask_d)

            # ---------- per-batch: main align, softmax, context ----------
            for b in range(BL):
                mes = []
                alm = small.tile([P, NT], f32, tag="alm")
                for c in range(NCH):
                    if b == 0:
                        met = b0_mes[c]
                    else:
                        met = mep.tile([P, TCH, D], ctx_dt, tag="me")
                        nc.sync.dma_start(out=met, in_=me_r[c, b].bitcast(ctx_dt))
                    mes.append(met)
                    for t in range(TCH):
                        dmp = dump.tile([P, D], f32, tag="dmp")
                        gt = c * TCH + t
                        nc.vector.scalar_tensor_tensor(
                            out=dmp, in0=met[:, t, :].bitcast(f32), scalar=1.0,
                            in1=QB[:, b, :], op0=AL.mult, op1=AL.mult,
                            accum_out=alm[:, gt : gt + 1],
                        )

                # masked scores: am[:,0,:]=main, am[:,1,:]=pos
                am = small.tile([P, 2, NT], f32, tag="am")
                nc.vector.tensor_tensor(out=am[:, 0, :], in0=alm,
                                        in1=mask_t[:, b, 0, :], op=AL.add)
                nc.vector.tensor_tensor(out=am[:, 1, :], in0=ALP[:, b, :],
                                        in1=mask_t[:, b, 1, :], op=AL.add)
                m2 = small.tile([P, 2], f32, tag="m2")
                nc.vector.tensor_reduce(out=m2, in_=am, axis=AX.X, op=AL.max)
                g2 = small.tile([P, 2], f32, tag="g2")
                nc.gpsimd.partition_all_reduce(
                    g2, m2, channels=P, reduce_op=bass_isa.ReduceOp.max
                )
                ng2 = small.tile([P, 2], f32, tag="ng2")
                nc.scalar.mul(ng2, g2, -1.0)

                ex = small.tile([P, 2, NT], f32, tag="ex")
                s3 = small.tile([P, 3], f32, tag="s3")
                nc.scalar.activation(
                    out=ex[:, 0, :], in_=am[:, 0, :], func=AF.Exp,
                    bias=ng2[:, 0:1], scale=1.0, accum_out=s3[:, 0:1],
                )
                nc.scalar.activation(
                    out=ex[:, 1, :], in_=am[:, 1, :], func=AF.Exp,
                    bias=ng2[:, 1:2], scale=1.0, accum_out=s3[:, 1:2],
                )
                avu = small.tile([P, NT], f32, tag="avu")
                nc.vector.scalar_tensor_tensor(
                    out=avu, in0=ex[:, 0, :], scalar=1.0, in1=ex[:, 1, :],
                    op0=AL.mult, op1=AL.mult, accum_out=s3[:, 2:3],
                )
                # context weights: unnormalized avu rounded to the context
                # dtype; normalization folded into the PSUM evacuation below
                avr = small.tile([P, NT], ctx_dt, tag="avr")
                nc.scalar.copy(avr, avu)

                z3 = small.tile([P, 3], f32, tag="z3")
                nc.gpsimd.partition_all_reduce(
                    z3, s3, channels=P, reduce_op=bass_isa.ReduceOp.add
                )
                r3 = small.tile([P, 3], f32, tag="r3")
                nc.vector.reciprocal(r3, z3)

                nc.scalar.activation(out=MAo[:, b, :], in_=ex[:, 0, :],
                                     func=AF.Copy, scale=r3[:, 0:1])
                nc.sync.dma_start(out=oma_d[:, b, :], in_=MAo[:, b, :])
                nc.scalar.activation(out=PAo[:, b, :], in_=ex[:, 1, :],
                                     func=AF.Copy, scale=r3[:, 1:2])
                nc.sync.dma_start(out=opa_d[:, b, :], in_=PAo[:, b, :])
                nc.scalar.activation(out=AVo[:, b, :], in_=avu,
                                     func=AF.Copy, scale=r3[:, 2:3])
                nc.sync.dma_start(out=oav_d[:, b, :], in_=AVo[:, b, :])

                # context: c_t = (1/Z) * sum_l avu[l] * me[l, :]
                ctx_p = ps_ctx.tile([1, D], f32, tag="ctx")
                for c in range(NCH):
                    for t in range(TCH):
                        gt = c * TCH + t
                        nc.tensor.matmul(
                            ctx_p,
                            lhsT=avr[:, gt : gt + 1],
                            rhs=mes[c][:, t, :],
                            start=(gt == 0), stop=(gt == NT - 1),
                        )
                cts = misc.tile([1, D], f32, tag="cts")
                nc.scalar.activation(out=cts, in_=ctx_p, func=AF.Copy,
                                     scale=r3[0:1, 2:3])
                for k in range(D // P):
                    tp_p = ps_tp.tile([P, 1], f32, tag="tp")
                    nc.tensor.transpose(
                        tp_p, cts[0:1, k * P : (k + 1) * P], ident1
                    )
                    nc.scalar.copy(CCT[:, k, b : b + 1], tp_p)

            # ---------- tail: attn_h.T = [c_t; s].T @ W_out.T ----------
            # concat chunks are the stationary operand (only 6/2 columns per
            # weight load); W_out.T streams as rhs; output is batch-major.
            with tc.tile_pool(name="ps_ah", bufs=2, space="PSUM") as psah:
                for lo, hi in ((0, 6), (6, BL)):
                    nb = hi - lo
                    ah_p = psah.tile([nb, D], f32, tag="ahp")
                    for jc in range(2 * D // P):
                        lhsT = (CCT[:, jc, lo:hi] if jc < D // P
                                else stR[:, jc - D // P, lo:hi])
                        nc.tensor.matmul(
                            ah_p, lhsT=lhsT, rhs=wt[:, jc, :],
                            start=(jc == 0), stop=(jc == 2 * D // P - 1),
                        )
                    ah_s = misc.tile([nb, D], f32, tag=f"ahs{lo}")
                    nc.scalar.activation(out=ah_s, in_=ah_p, func=AF.Tanh)
                    nc.sync.dma_start(out=oah_d[lo:hi, :], in_=ah_s)

    nc.compile()
    return nc


def _get_nc():
    if "nc" not in _CACHE:
        _CACHE["nc"] = _build()
    return _CACHE["nc"]


def _make_in_maps(source, main_embs, pos_embs, W_main_in, W_pos_in, W_out,
                  memory_lengths):
    source = np.asarray(source, np.float32)
    main_embs = np.asarray(main_embs, np.float32)
    pos_embs = np.asarray(pos_embs, np.float32)
    W_main_in = np.ascontiguousarray(np.asarray(W_main_in, np.float32))
    W_pos_in = np.ascontiguousarray(np.asarray(W_pos_in, np.float32))
    woutT = np.ascontiguousarray(np.asarray(W_out, np.float32).T)
    lens = np.asarray(memory_lengths)

    maskadd = np.where(
        np.arange(L)[None, :] < np.asarray(lens, np.int64)[:, None], 0.0, -1e30
    ).astype(np.float32)                      # (B, L)
    m = maskadd.reshape(B, NT, P).transpose(2, 0, 1)  # (P, B, NT)
    mask2 = np.ascontiguousarray(np.stack([m, m], axis=2))  # (P, B, 2, NT)

    in_maps = []
    for c in range(NCORES):
        sl = slice(c * BL, (c + 1) * BL)
        srcT = np.ascontiguousarray(
            source[sl].T.reshape(D // P, P, BL).transpose(1, 0, 2)
        )  # (P, 4, BL): srcT[p,h,b] = source[c*BL+b, h*P+p]
        in_maps.append({
            "me": np.ascontiguousarray(main_embs[:, sl, :]),
            "pos": np.ascontiguousarray(pos_embs[:, sl, :]),
            "srcT": srcT,
            "wmain": W_main_in,
            "wpos": W_pos_in,
            "woutT": woutT,
            "mask2": np.ascontiguousarray(mask2[:, sl]),
        })
    return in_maps


def _assemble(results):
    ah_rows, ma_rows, pa_rows, av_rows = [], [], [], []
    for r in results:
        ah_rows.append(np.asarray(r["out_ah"]))
        for rows, key in ((ma_rows, "out_ma"), (pa_rows, "out_pa"),
                          (av_rows, "out_av")):
            # (P, BL, NT): x[b, t*P+p] = arr[p, b, t]
            rows.append(np.ascontiguousarray(
                np.asarray(r[key]).transpose(1, 2, 0).reshape(BL, L)))
    attn_h = np.concatenate(ah_rows, 0)
    ma = np.concatenate(ma_rows, 0)
    pa = np.concatenate(pa_rows, 0)
    av = np.concatenate(av_rows, 0)
    return attn_h, av, ma, pa


def run_hw(inputs, trace=False, **kw):
    from concourse import bass_utils
    nc = _get_nc()
    in_maps = _make_in_maps(**inputs)
    res = bass_utils.run_bass_kernel_spmd(
        nc, in_maps, core_ids=list(range(NCORES)), trace=trace, **kw
    )
    return _assemble(res.results), res


def kernel(source, main_embs, pos_embs, W_main_in, W_pos_in, W_out,
           memory_lengths):
    (attn_h, av, ma, pa), _ = run_hw(dict(
        source=source, main_embs=main_embs, pos_embs=pos_embs,
        W_main_in=W_main_in, W_pos_in=W_pos_in, W_out=W_out,
        memory_lengths=memory_lengths,
    ))
    return attn_h, av, ma, pa
